# revision 1
# baseline (speedup 1.0000x reference)
"""Trainium2 Bass kernel for nn_CrossAttention_72275709657317.

Reference computation (B=4, S=2048, E=1024, D=64):
    Q = x @ Wq.T + bq                      [B,S,D]
    K = y @ Wk.T + bk                      [B,S,D]
    scores = Q @ K.T / sqrt(D)             [B,Sq,Sk]
    attn = softmax(scores, axis=1)         (softmax over the QUERY axis)
    V = (y @ WvR.T + bvR) @ WvL.T + bvL    [B,S,E]
    out = attn @ V                         [B,S,E]

Key algebraic restructuring:
  * V is rank-64 (+bias), so attn @ V = (attn @ [VR | 1]) @ [[WvL.T],[bvL]]
    -- the dominant S*S*E matmul collapses to S*S*D.
  * softmax over q: attn[q,k] = exp(s[q,k])/den[k], den[k] = sum_q exp(s[q,k]).
    den only enters per-k, so it is folded into the VR' rows; attnT itself is
    kept unnormalized.

Sharding: 8 cores -> (batch b = c//2, query-half h = c%2). Each core computes
K/VR projections for its local k-half; the pair exchanges them (and the exp
column-sum partials) via pairwise AllReduce.  All cross-core data uses the
"partner = pair_sum - mine" identity so the single SPMD program is h-agnostic.

Matmuls run in fp32r (full PE speed, ~1.5e-4 rel err).  HW quirk: fp32r
ACCUMULATING chains require the full 128-wide lhsT free dim (M<128 chains hang
the exec unit), so K/VR projections are fused into one M=128 chain (psum rows
0:64 = K^T, 64:128 = VR^T), the Q chain duplicates Wq, and VR' is zero-padded
to M=128 for the O1 chain.
"""
import numpy as np

import concourse.bass as bass
import concourse.tile as tile
from concourse import bacc, mybir
from concourse.masks import make_identity
from concourse.bass_utils import run_bass_kernel_spmd

N_CORES = 8
B, S, E, D = 4, 2048, 1024, 64
H = S // 2            # per-core q rows / local k rows
P = 128
EB = E // P           # 8 e-chunks
BLK = 256             # s-rows per transpose/projection block
BCH = BLK // P        # 2
NBLK = H // BLK       # 4
KC = S // P           # 16 k-chunks
KCL = H // P          # 8 local k-chunks
NQ = H // 512         # 2 q-chunks of 512
DV = D + 1            # VR plus folded-ones column
F32 = mybir.dt.float32
F32R = mybir.dt.float32r
EXP = mybir.ActivationFunctionType.Exp
ADD = mybir.AluOpType.add
GROUPS = [[0, 1], [2, 3], [4, 5], [6, 7]]

IN_SPECS = [
    ("x", [H, E]), ("y", [H, E]),
    ("Wq", [D, E]), ("bq", [D]), ("Wk", [D, E]), ("bk", [D]),
    ("WvR", [D, E]), ("bvR", [D]), ("WvL", [E, D]), ("bvL", [E]),
]


def _emit(tc, aps, out_ap, no_cc=False, no_accum=False, stop_stage=99):
    nc = tc.nc
    from contextlib import ExitStack
    with ExitStack() as ctx:
        const = ctx.enter_context(tc.tile_pool(name="const", bufs=1))
        io = ctx.enter_context(tc.tile_pool(name="io", bufs=3))
        tb = ctx.enter_context(tc.tile_pool(name="tb", bufs=2))
        work = ctx.enter_context(tc.tile_pool(name="work", bufs=2))
        big = ctx.enter_context(tc.tile_pool(name="big", bufs=1))
        tp_ps = ctx.enter_context(tc.tile_pool(name="tp_ps", bufs=3, space="PSUM"))
        mm_ps = ctx.enter_context(tc.tile_pool(name="mm_ps", bufs=2, space="PSUM"))
        o1_ps = ctx.enter_context(tc.tile_pool(name="o1_ps", bufs=2, space="PSUM"))
        dram = ctx.enter_context(tc.tile_pool(name="dram", bufs=1, space="DRAM"))

        # ---------------- constants ----------------
        ident = const.tile([P, P], F32)
        make_identity(nc, ident[:])
        zeros64 = const.tile([P, D], F32)
        nc.gpsimd.memset(zeros64[:], 0.0)

        if stop_stage <= -3:
            _early = const.tile([P, P], F32, name="early")
            nc.vector.tensor_copy(_early[:], ident[:])
            nc.sync.dma_start(out_ap[0:P, 0:P], _early[:])
            return

        # biases: contiguous [1,64] rows -> one PE transpose -> per-partition cols
        bias_stage = const.tile([P, D], F32)
        nc.sync.dma_start(bias_stage[0:1, :], aps["bk"].rearrange("(o f) -> o f", o=1))
        nc.sync.dma_start(bias_stage[1:2, :], aps["bvR"].rearrange("(o f) -> o f", o=1))
        nc.sync.dma_start(bias_stage[2:3, :], aps["bq"].rearrange("(o f) -> o f", o=1))
        bias_ps = tp_ps.tile([P, 4 * P], F32, name="tp4")
        nc.tensor.transpose(bias_ps[0:D, 0:P], bias_stage[:], ident[:])
        bias_kv = const.tile([P, 1], F32)
        nc.vector.tensor_copy(bias_kv[0:D, :], bias_ps[0:D, 0:1])
        nc.vector.tensor_copy(bias_kv[D:P, :], bias_ps[0:D, 1:2])
        bias_q = const.tile([D, 1], F32)
        nc.vector.tensor_copy(bias_q[:], bias_ps[0:D, 2:3])

        if stop_stage <= -2:
            _early = const.tile([P, 1], F32, name="early2")
            nc.vector.tensor_copy(_early[:], bias_kv[:])
            nc.sync.dma_start(out_ap[0:P, 0:1], _early[:])
            return

        # fused lhsT weights: WkvT[:, ei, 0:64] = Wk^T, [:, ei, 64:128] = WvR^T
        # WqqT duplicates Wq^T into both halves (fp32r chains need M=128).
        def build_fused_wT(name, src_lo, src_hi):
            wt = const.tile([P, EB, P], F32R, name=name)
            for half, src in ((0, src_lo), (1, src_hi)):
                stage = const.tile([P, E], F32, name=f"stage_{name}_{half}")
                nc.gpsimd.memset(stage[:], 0.0)
                nc.sync.dma_start(stage[0:D, :], aps[src])
                for g in range(2):
                    ps = tp_ps.tile([P, 4 * P], F32, name="tp4")
                    for j in range(4):
                        ei = 4 * g + j
                        nc.tensor.transpose(ps[:, j * P:(j + 1) * P],
                                            stage[:, ei * P:(ei + 1) * P], ident[:])
                    nc.vector.tensor_copy(
                        wt[:, 4 * g:4 * g + 4, half * D:half * D + D],
                        ps[:].rearrange("p (a b) -> p a b", a=4)[:, :, 0:D])
            return wt

        WkvT = build_fused_wT("WkvT", "Wk", "WvR")
        WqqT = build_fused_wT("WqqT", "Wq", "Wq")

        if stop_stage <= -1:
            nc.sync.dma_start(out_ap[0:P, 0:P], WqqT[:, 0, :].bitcast(F32))
            return

        # WvLT: [DV, E] fp32r; row D = bvL
        wvls = const.tile([P, EB, D], F32)
        for vo in range(EB):
            nc.sync.dma_start(wvls[:, vo, :], aps["WvL"][vo * P:(vo + 1) * P, :])
        WvLT = const.tile([DV, E], F32R)
        for g in range(2):
            ps = tp_ps.tile([P, 4 * P], F32, name="tp4")
            for j in range(4):
                vo = 4 * g + j
                nc.tensor.transpose(ps[0:D, j * P:(j + 1) * P], wvls[:, vo, :], ident[:])
            nc.vector.tensor_copy(WvLT[0:D, g * 512:(g + 1) * 512], ps[0:D, :])
        bvls = const.tile([1, E], F32)
        nc.sync.dma_start(bvls[:], aps["bvL"].rearrange("(o f) -> o f", o=1))
        nc.vector.tensor_copy(WvLT[D:DV, :], bvls[:])

        # ---------------- persistent tiles ----------------
        KT = big.tile([D, S], F32R, name="KT")         # [64, 2048] scores lhsT
        QT = big.tile([D, H], F32R, name="QT")         # [64, 1024] scores rhs
        KTVR_l = big.tile([P, H], F32, name="KTVR_l")  # rows 0:64 K^T, 64:128 VR^T
        attnT = big.tile([P, KC, H], F32R, name="attnT")
        den2 = big.tile([P, KC, NQ], F32, name="den2")

        kv_loc = dram.tile([P, H], F32)
        kv_sum = dram.tile([P, H], F32)
        den_dram = dram.tile([P, KC], F32)
        den_sum_dram = dram.tile([P, KC], F32)

        def _dump_and_stop(tile_ap, rows, cols):
            nc.sync.dma_start(out_ap[0:rows, 0:cols], tile_ap)

        if stop_stage <= 0:
            _dump_and_stop(ident[:], P, P)
            return

        # ---------------- projection block pipeline ----------------
        def proj_blocks(src_ap, wt, bias, dst_fn, dst_rows):
            for blk in range(NBLK):
                xb = io.tile([P, BCH, E], F32, name="inblk")
                nc.sync.dma_start(
                    xb[:],
                    src_ap[blk * BLK:(blk + 1) * BLK, :]
                    .rearrange("(c p) e -> p c e", p=P))
                xT = tb.tile([P, EB, BLK], F32R, name="tblk")
                for c in range(BCH):
                    for g in range(2):
                        ps = tp_ps.tile([P, 4 * P], F32, name="tp4")
                        for j in range(4):
                            ei = 4 * g + j
                            nc.tensor.transpose(ps[:, j * P:(j + 1) * P],
                                                xb[:, c, ei * P:(ei + 1) * P],
                                                ident[:])
                        nc.vector.tensor_copy(
                            xT[:, 4 * g:4 * g + 4, c * P:(c + 1) * P],
                            ps[:].rearrange("p (a b) -> p a b", a=4))
                ps = mm_ps.tile([P, 512], F32, name="mmps")
                for ei in range(EB):
                    nc.tensor.matmul(ps[:, 0:BLK], wt[:, ei, :], xT[:, ei, :],
                                     start=(ei == 0), stop=(ei == EB - 1))
                nc.scalar.add(dst_fn(blk), ps[0:dst_rows, 0:BLK], bias[:])

        # y and x paths interleaved: earlier QT availability for local scores
        proj_blocks(aps["y"], WkvT, bias_kv,
                    lambda blk: KTVR_l[:, blk * BLK:(blk + 1) * BLK], P)

        if stop_stage <= 1:
            _dump_and_stop(KTVR_l[0:D, :], D, H)
            return

        # collective 1: exchange K^T / VR^T within the pair
        nc.sync.dma_start(kv_loc[:], KTVR_l[:])
        if no_cc:
            nc.sync.dma_start(kv_sum[:], kv_loc[:])
        else:
            nc.gpsimd.collective_compute(
                "AllReduce", ADD, replica_groups=GROUPS,
                ins=[kv_loc.opt()], outs=[kv_sum.opt()])
        kvs = big.tile([P, H], F32, name="kvs")
        nc.sync.dma_start(kvs[:], kv_sum[:])
        KTVR_r = big.tile([P, H], F32, name="KTVR_r")
        nc.vector.tensor_sub(KTVR_r[:], kvs[:], KTVR_l[:])   # partner = sum - mine
        nc.vector.tensor_copy(KT[:, 0:H], KTVR_l[0:D, :])    # rounded to fp32r
        nc.vector.tensor_copy(KT[:, H:S], KTVR_r[0:D, :])

        if stop_stage <= 2:
            _dump_and_stop(KT[:, 0:H].bitcast(F32), D, H)
            return

        # x path: Q^T (overlaps collective 1)
        proj_blocks(aps["x"], WqqT, bias_q,
                    lambda blk: QT[:, blk * BLK:(blk + 1) * BLK], D)

        if stop_stage <= 3:
            _dump_and_stop(QT[:].bitcast(F32), D, H)
            return

        # ---------------- scoresT + exp + den partials ----------------
        for kc in range(KC):
            for qc in range(NQ):
                sps = mm_ps.tile([P, 512], F32, name="mmps")
                nc.tensor.matmul(sps[:], KT[:, kc * P:(kc + 1) * P],
                                 QT[:, qc * 512:(qc + 1) * 512],
                                 start=True, stop=True)
                nc.scalar.activation(attnT[:, kc, qc * 512:(qc + 1) * 512], sps[:],
                                     EXP, scale=0.125,
                                     accum_out=None if no_accum else den2[:, kc, qc:qc + 1])

        if stop_stage <= 4:
            _dump_and_stop(attnT[:, 0, :].bitcast(F32), P, H)
            return

        # ---------------- VR unscaled transposes (overlap exp/den) ----------
        VRu = big.tile([P, KC, D], F32, name="VRu")
        VRp = big.tile([P, KC, P], F32R, name="VRp")
        for g in range(KC // 4):
            ps = tp_ps.tile([P, 4 * P], F32, name="tp4")
            for j in range(4):
                kc = 4 * g + j
                src_t = KTVR_l if kc < KCL else KTVR_r
                col = (kc if kc < KCL else kc - KCL) * P
                nc.tensor.transpose(ps[:, j * P:(j + 1) * P],
                                    src_t[:, col:col + P], ident[:])
            for j in range(4):
                kc = 4 * g + j
                nc.vector.tensor_copy(VRu[:, kc, :], ps[:, j * P + D:(j + 1) * P])
                nc.vector.tensor_copy(VRp[:, kc, DV:P], zeros64[:, 0:P - DV])

        # ---------------- den exchange + reciprocal ----------------
        den_loc = big.tile([P, KC], F32, name="den_loc")
        if no_accum:
            for kc in range(KC):
                nc.vector.tensor_reduce(den_loc[:, kc:kc + 1],
                                        attnT[:, kc, :].bitcast(F32),
                                        axis=mybir.AxisListType.X, op=ADD)
        else:
            nc.vector.tensor_reduce(den_loc[:], den2[:], axis=mybir.AxisListType.X, op=ADD)
        nc.sync.dma_start(den_dram[:], den_loc[:])
        if no_cc:
            nc.sync.dma_start(den_sum_dram[:], den_dram[:])
        else:
            nc.gpsimd.collective_compute(
                "AllReduce", ADD, replica_groups=GROUPS,
                ins=[den_dram.opt()], outs=[den_sum_dram.opt()])
        dsum = big.tile([P, KC], F32, name="dsum")
        nc.sync.dma_start(dsum[:], den_sum_dram[:])
        partner = big.tile([P, KC], F32, name="partner")
        nc.vector.tensor_sub(partner[:], dsum[:], den_loc[:])
        denf = big.tile([P, KC], F32, name="denf")
        # my chunk order is [local | remote]; partner's is swapped
        nc.vector.tensor_add(denf[:, 0:KCL], den_loc[:, 0:KCL], partner[:, KCL:KC])
        nc.vector.tensor_add(denf[:, KCL:KC], den_loc[:, KCL:KC], partner[:, 0:KCL])
        r_sb = big.tile([P, KC], F32, name="r_sb")
        nc.vector.reciprocal(r_sb[:], denf[:])

        if stop_stage <= 5:
            _dump_and_stop(r_sb[:], P, KC)
            return

        # ---------------- VR' = [VR * r | r | 0-pad] ----------------
        for kc in range(KC):
            nc.vector.tensor_scalar_mul(VRp[:, kc, 0:D], VRu[:, kc, :],
                                        r_sb[:, kc:kc + 1])
            nc.vector.tensor_copy(VRp[:, kc, D:DV], r_sb[:, kc:kc + 1])

        if stop_stage <= 6:
            _dump_and_stop(VRp[:, 0, :].bitcast(F32), P, P)
            return

        # ---------------- O1T = VR'^T @ attnT ----------------
        O1T = big.tile([DV, H], F32R, name="O1T")
        for qc in range(NQ):
            ops_ = o1_ps.tile([P, 512], F32, name="o1ps")
            for kc in range(KC):
                nc.tensor.matmul(ops_[:], VRp[:, kc, :],
                                 attnT[:, kc, qc * 512:(qc + 1) * 512],
                                 start=(kc == 0), stop=(kc == KC - 1))
            nc.scalar.copy(O1T[:, qc * 512:(qc + 1) * 512], ops_[0:DV, :])

        if stop_stage <= 7:
            _dump_and_stop(O1T[:].bitcast(F32), DV, H)
            return

        # ---------------- out = O1T^T @ WvL'T ----------------
        for qo in range(H // P):
            ot = work.tile([P, E], F32, name="outt")
            for vc in range(2):
                fps = mm_ps.tile([P, 512], F32, name="mmps")
                nc.tensor.matmul(fps[:], O1T[:, qo * P:(qo + 1) * P],
                                 WvLT[:, vc * 512:(vc + 1) * 512],
                                 start=True, stop=True)
                nc.vector.tensor_copy(ot[:, vc * 512:(vc + 1) * 512], fps[:])
            nc.sync.dma_start(out_ap[qo * P:(qo + 1) * P, :], ot[:])


def build_nc(reps: int = 1, no_cc=False, no_accum=False, stop_stage=99):
    nc = bacc.Bacc("TRN2", target_bir_lowering=False, debug=False,
                   num_devices=N_CORES)
    aps = {name: nc.dram_tensor(name, shape, F32, kind="ExternalInput").ap()
           for name, shape in IN_SPECS}
    out_ap = nc.dram_tensor("out", [H, E], F32, kind="ExternalOutput").ap()
    with tile.TileContext(nc) as tc:
        if reps == 1:
            _emit(tc, aps, out_ap, no_cc=no_cc, no_accum=no_accum, stop_stage=stop_stage)
        else:
            with tc.For_i(0, reps, 1):
                _emit(tc, aps, out_ap, no_cc=no_cc, no_accum=no_accum, stop_stage=stop_stage)
    nc.compile()
    return nc


def make_in_maps(inputs):
    arrs = {k: np.ascontiguousarray(np.asarray(v, dtype=np.float32))
            for k, v in inputs.items()}
    in_maps = []
    for c in range(N_CORES):
        b, h = divmod(c, 2)
        m = {"x": np.ascontiguousarray(arrs["x"][b, h * H:(h + 1) * H, :]),
             "y": np.ascontiguousarray(arrs["y"][b, h * H:(h + 1) * H, :])}
        for wn in ("Wq", "bq", "Wk", "bk", "WvR", "bvR", "WvL", "bvL"):
            m[wn] = arrs[wn]
        in_maps.append(m)
    return in_maps


def assemble_out(results):
    out = np.empty((B, S, E), dtype=np.float32)
    for c in range(N_CORES):
        b, h = divmod(c, 2)
        out[b, h * H:(h + 1) * H, :] = results[c]["out"]
    return out


_NC = None


def kernel(**inputs) -> np.ndarray:
    global _NC
    if _NC is None:
        _NC = build_nc()
    in_maps = make_in_maps(inputs)
    res = run_bass_kernel_spmd(_NC, in_maps, list(range(N_CORES)))
    return assemble_out(res.results)



# revision 3
# speedup vs baseline: 1.1957x; 1.1957x over previous
"""Trainium2 Bass kernel for nn_CrossAttention_72275709657317 (v2: bf16).

Reference computation (B=4, S=2048, E=1024, D=64):
    Q = x @ Wq.T + bq                      [B,S,D]
    K = y @ Wk.T + bk                      [B,S,D]
    scores = Q @ K.T / sqrt(D)             [B,Sq,Sk]
    attn = softmax(scores, axis=1)         (softmax over the QUERY axis)
    V = (y @ WvR.T + bvR) @ WvL.T + bvL    [B,S,E]
    out = attn @ V                         [B,S,E]

Algebraic restructuring (as v1):
  * V is rank-64 (+bias): attn @ V = (attn @ [VR | 1]) @ [[WvL.T],[bvL]]
  * softmax over q: attn[q,k] = exp(s[q,k])/den[k]; den folded into VR' rows.

v2 changes vs v1 (fp32r):
  * All matmul operands are bf16: fp32 moving operands stream at 2 cycles/col
    on the PE, bf16 at 1 — halves every matmul; bf16 lhsT also gets FWL.
    PSUM accumulation stays fp32; tolerance budget is 2e-2.
  * Inputs cast f32->bf16 during the DMA load (gpsimd SWDGE cast).
  * bf16 PE transposes write bf16 PSUM -> packed 2x DVE copies.
  * K/VR pair exchange is a bf16 AllGather in rank order (no sum-subtract);
    den columns then align across the pair so den is a plain AllReduce.
  * FD=1024 exp over two-bank PSUM tiles; FD=1024 copies.

Sharding: 8 cores -> (batch b = c//2, seq-half h = c%2). Core computes its
q-half of scores/out and its k-half of K/VR; pairs exchange K/VR + den.
"""
import numpy as np

import concourse.bass as bass
import concourse.tile as tile
from concourse import bacc, mybir
from concourse.masks import make_identity
from concourse.bass_utils import run_bass_kernel_spmd

N_CORES = 8
B, S, E, D = 4, 2048, 1024, 64
H = S // 2            # per-core q rows / local k rows
P = 128
EB = E // P           # 8 e-chunks
SB = H // P           # 8 s-chunks
KC = S // P           # 16 global k-chunks
DV = D + 1            # VR plus folded-ones column
F32 = mybir.dt.float32
BF16 = mybir.dt.bfloat16
EXP = mybir.ActivationFunctionType.Exp
ADD = mybir.AluOpType.add
BYPASS = mybir.AluOpType.bypass
GROUPS = [[0, 1], [2, 3], [4, 5], [6, 7]]

IN_SPECS = [
    ("x", [H, E]), ("y", [H, E]),
    ("Wq", [D, E]), ("bq", [D]), ("Wk", [D, E]), ("bk", [D]),
    ("WvR", [D, E]), ("bvR", [D]), ("WvL", [E, D]), ("bvL", [E]),
]


def _emit(tc, aps, out_ap, no_cc=False, no_accum=False, stop_stage=99):
    nc = tc.nc
    from contextlib import ExitStack
    with ExitStack() as ctx:
        const = ctx.enter_context(tc.tile_pool(name="const", bufs=1))
        big = ctx.enter_context(tc.tile_pool(name="big", bufs=1))
        work = ctx.enter_context(tc.tile_pool(name="work", bufs=2))
        tp_ps = ctx.enter_context(tc.tile_pool(name="tp_ps", bufs=2, space="PSUM"))
        mm_ps = ctx.enter_context(tc.tile_pool(name="mm_ps", bufs=2, space="PSUM"))
        sc_ps = ctx.enter_context(tc.tile_pool(name="sc_ps", bufs=2, space="PSUM"))
        dram = ctx.enter_context(tc.tile_pool(name="dram", bufs=1, space="DRAM"))

        # ---------------- identities ----------------
        ident = const.tile([P, P], F32)
        make_identity(nc, ident[:])
        identb = const.tile([P, P], BF16)
        make_identity(nc, identb[:])

        # ---------------- input loads (cast f32->bf16 in DMA) ----------
        Wk_bf = const.tile([D, E], BF16)
        WvR_bf = const.tile([D, E], BF16)
        Wq_bf = const.tile([D, E], BF16)
        WvL_bf = const.tile([P, EB, D], BF16)
        y_bf = big.tile([P, SB, E], BF16, name="y_bf")
        x_bf = big.tile([P, SB, E], BF16, name="x_bf")
        nc.gpsimd.dma_start(Wk_bf[:], aps["Wk"])
        nc.gpsimd.dma_start(WvR_bf[:], aps["WvR"])
        nc.gpsimd.dma_start(Wq_bf[:], aps["Wq"])
        nc.gpsimd.dma_start(WvL_bf[:], aps["WvL"].rearrange("(vo p) d -> p vo d", p=P))
        for sb2 in range(SB // 2):
            nc.gpsimd.dma_start(
                y_bf[:, 2 * sb2:2 * sb2 + 2, :],
                aps["y"][sb2 * 256:(sb2 + 1) * 256, :]
                .rearrange("(c p) e -> p c e", p=P))
        for sb2 in range(SB // 2):
            nc.gpsimd.dma_start(
                x_bf[:, 2 * sb2:2 * sb2 + 2, :],
                aps["x"][sb2 * 256:(sb2 + 1) * 256, :]
                .rearrange("(c p) e -> p c e", p=P))

        # ---------------- biases (f32, one transpose) ----------------
        bias_stage = const.tile([P, D], F32)
        nc.sync.dma_start(bias_stage[0:1, :], aps["bk"].rearrange("(o f) -> o f", o=1))
        nc.sync.dma_start(bias_stage[1:2, :], aps["bvR"].rearrange("(o f) -> o f", o=1))
        nc.sync.dma_start(bias_stage[2:3, :], aps["bq"].rearrange("(o f) -> o f", o=1))
        bias_ps = mm_ps.tile([P, 4 * P], F32, name="mmps")
        nc.tensor.transpose(bias_ps[0:D, 0:P], bias_stage[:], ident[:])
        bias_kv = const.tile([P, 1], F32)
        nc.vector.tensor_copy(bias_kv[0:D, :], bias_ps[0:D, 0:1])
        nc.vector.tensor_copy(bias_kv[D:P, :], bias_ps[0:D, 1:2])
        bias_q = const.tile([D, 1], F32)
        nc.vector.tensor_copy(bias_q[:], bias_ps[0:D, 2:3])

        # ---------------- fused lhsT weights (bf16) ----------------
        # WkvT[:, ei, 0:64] = Wk^T, [:, ei, 64:128] = WvR^T
        WkvT = const.tile([P, EB, P], BF16)
        ps_kv = tp_ps.tile([P, 8 * P], BF16, name="tpb")
        for ei in range(EB):
            nc.tensor.transpose(ps_kv[:, ei * P:ei * P + D],
                                Wk_bf[:, ei * P:(ei + 1) * P], identb[0:D, 0:D])
            nc.tensor.transpose(ps_kv[:, ei * P + D:(ei + 1) * P],
                                WvR_bf[:, ei * P:(ei + 1) * P], identb[0:D, 0:D])
        nc.vector.tensor_copy(WkvT[:], ps_kv[:].rearrange("p (a b) -> p a b", a=EB))

        # WqqT duplicates Wq^T into both halves (M=128 chain)
        WqqT = const.tile([P, EB, P], BF16)
        ps_q = tp_ps.tile([P, 8 * P], BF16, name="tpb")
        for ei in range(EB):
            nc.tensor.transpose(ps_q[:, ei * P:ei * P + D],
                                Wq_bf[:, ei * P:(ei + 1) * P], identb[0:D, 0:D])
        psq3 = ps_q[:].rearrange("p (a b) -> p a b", a=EB)
        nc.vector.tensor_copy(WqqT[:, :, 0:D], psq3[:, :, 0:D])
        nc.vector.tensor_copy(WqqT[:, :, D:P], psq3[:, :, 0:D])

        # WvLT: rows 0:64 = WvL^T, row 64 = bvL
        WvLT = const.tile([DV, E], BF16)
        ps_v = tp_ps.tile([P, 8 * P], BF16, name="tpb")
        for vo in range(EB):
            nc.tensor.transpose(ps_v[0:D, vo * P:(vo + 1) * P],
                                WvL_bf[:, vo, :], identb[:])
        nc.vector.tensor_copy(WvLT[0:D, :], ps_v[0:D, :])
        nc.gpsimd.dma_start(WvLT[D:DV, :], aps["bvL"].rearrange("(o f) -> o f", o=1))

        # ---------------- persistent tiles ----------------
        yT = big.tile([P, EB, H], BF16, name="yT")
        xT = big.tile([P, EB, H], BF16, name="xT")
        QT = big.tile([D, H], BF16, name="QT")
        KTVR_l = big.tile([P, H], BF16, name="KTVR_l")
        KTVR = big.tile([P, 2, H], BF16, name="KTVR")   # rank-ordered pair
        attnT = big.tile([P, KC, H], BF16, name="attnT")
        den = big.tile([P, KC], F32, name="den")
        dsum = big.tile([P, KC], F32, name="dsum")
        r_sb = big.tile([P, KC], F32, name="r_sb")
        VRu = big.tile([P, KC, D], BF16, name="VRu")
        VRp = big.tile([P, KC, P], BF16, name="VRp")
        O1T = big.tile([DV, H], BF16, name="O1T")

        kv_loc = dram.tile([P, H], BF16)
        kv_all = dram.tile([2, P, H], BF16)
        den_dram = dram.tile([P, KC], F32)
        den_sum_dram = dram.tile([P, KC], F32)

        nc.gpsimd.memset(VRp[:], 0.0)   # zero-pad cols DV:P for the O1 chain

        if stop_stage <= 0:
            nc.sync.dma_start(out_ap[0:P, 0:P], ident[:])
            return

        # ---------------- transpose + projection blocks ----------------
        def trans_block(src_bf, dstT, sb):
            ps = tp_ps.tile([P, 8 * P], BF16, name="tpb")
            for ei in range(EB):
                nc.tensor.transpose(ps[:, ei * P:(ei + 1) * P],
                                    src_bf[:, sb, ei * P:(ei + 1) * P], identb[:])
            nc.vector.tensor_copy(dstT[:, :, sb * P:(sb + 1) * P],
                                  ps[:].rearrange("p (a b) -> p a b", a=EB))

        def proj_half(wT, srcT, h2, dst_fn):
            ps = mm_ps.tile([P, 512], F32, name="mmps")
            for ei in range(EB):
                nc.tensor.matmul(ps[:], wT[:, ei, :],
                                 srcT[:, ei, h2 * 512:(h2 + 1) * 512],
                                 start=(ei == 0), stop=(ei == EB - 1))
            dst_fn(ps)

        # y path -> KTVR_l
        for sb in range(4):
            trans_block(y_bf, yT, sb)
        proj_half(WkvT, yT, 0,
                  lambda ps: nc.scalar.add(KTVR_l[:, 0:512], ps[:], bias_kv[:]))
        for sb in range(4, SB):
            trans_block(y_bf, yT, sb)
        proj_half(WkvT, yT, 1,
                  lambda ps: nc.scalar.add(KTVR_l[:, 512:1024], ps[:], bias_kv[:]))

        if stop_stage <= 1:
            nc.sync.dma_start(out_ap[0:P, 0:512], KTVR_l[:].bitcast(F32))
            return

        # pair exchange: AllGather K^T/VR^T in rank order
        nc.sync.dma_start(kv_loc[:], KTVR_l[:])
        if no_cc:
            nc.sync.dma_start(kv_all[0], kv_loc[:])
            nc.sync.dma_start(kv_all[1], kv_loc[:])
        else:
            nc.gpsimd.collective_compute(
                "AllGather", BYPASS, replica_groups=GROUPS,
                ins=[kv_loc.opt()], outs=[kv_all.opt()])
        nc.sync.dma_start(KTVR[:], kv_all[:].rearrange("r p h -> p r h"))

        if stop_stage <= 2:
            nc.sync.dma_start(out_ap[0:P, 0:512], KTVR[:, 0, :].bitcast(F32))
            return

        # x path -> QT (overlaps the collective)
        for sb in range(4):
            trans_block(x_bf, xT, sb)
        proj_half(WqqT, xT, 0,
                  lambda ps: nc.scalar.add(QT[:, 0:512], ps[0:D, :], bias_q[:]))
        for sb in range(4, SB):
            trans_block(x_bf, xT, sb)
        proj_half(WqqT, xT, 1,
                  lambda ps: nc.scalar.add(QT[:, 512:1024], ps[0:D, :], bias_q[:]))

        if stop_stage <= 3:
            nc.sync.dma_start(out_ap[0:D, 0:512], QT[:].bitcast(F32))
            return

        # ---------------- scoresT + exp + den partials ----------------
        for kc in range(KC):
            r, j = divmod(kc, SB)
            sps = sc_ps.tile([P, 1024], F32, name="scps")
            for qc in range(2):
                nc.tensor.matmul(sps[:, qc * 512:(qc + 1) * 512],
                                 KTVR[0:D, r, j * P:(j + 1) * P],
                                 QT[:, qc * 512:(qc + 1) * 512],
                                 start=True, stop=True)
            nc.scalar.activation(attnT[:, kc, :], sps[:], EXP, scale=0.125,
                                 accum_out=den[:, kc:kc + 1])

        # VR^T -> VRu (overlaps exp on ACT)
        for g in range(2):
            ps = tp_ps.tile([P, 8 * P], BF16, name="tpb")
            for j in range(SB):
                nc.tensor.transpose(ps[:, j * P:(j + 1) * P],
                                    KTVR[:, g, j * P:(j + 1) * P], identb[:])
            nc.vector.tensor_copy(
                VRu[:, 8 * g:8 * g + 8, :],
                ps[:].rearrange("p (a b) -> p a b", a=SB)[:, :, D:P])

        if stop_stage <= 4:
            nc.sync.dma_start(out_ap[0:P, 0:512], attnT[:, 0, :].bitcast(F32))
            return

        # ---------------- den exchange + reciprocal ----------------
        nc.sync.dma_start(den_dram[:], den[:])
        if no_cc:
            nc.sync.dma_start(den_sum_dram[:], den_dram[:])
        else:
            nc.gpsimd.collective_compute(
                "AllReduce", ADD, replica_groups=GROUPS,
                ins=[den_dram.opt()], outs=[den_sum_dram.opt()])
        nc.sync.dma_start(dsum[:], den_sum_dram[:])
        nc.vector.reciprocal(r_sb[:], dsum[:])

        if stop_stage <= 5:
            nc.sync.dma_start(out_ap[0:P, 0:KC], r_sb[:])
            return

        # ---------------- VR' = [VR * r | r | 0-pad] ----------------
        for kc in range(KC):
            nc.vector.tensor_scalar_mul(VRp[:, kc, 0:D], VRu[:, kc, :],
                                        r_sb[:, kc:kc + 1])
        r_bf = big.tile([P, KC], BF16, name="r_bf")
        nc.vector.tensor_copy(r_bf[:], r_sb[:])
        nc.vector.tensor_copy(VRp[:, :, D:DV], r_bf[:].rearrange("p (a b) -> p a b", a=KC))

        if stop_stage <= 6:
            nc.sync.dma_start(out_ap[0:P, 0:64], VRp[:, 0, :].bitcast(F32))
            return

        # ---------------- O1T = VR'^T @ attnT ----------------
        for qc in range(2):
            ops_ = mm_ps.tile([P, 512], F32, name="mmps")
            for kc in range(KC):
                nc.tensor.matmul(ops_[:], VRp[:, kc, :],
                                 attnT[:, kc, qc * 512:(qc + 1) * 512],
                                 start=(kc == 0), stop=(kc == KC - 1))
            nc.scalar.copy(O1T[:, qc * 512:(qc + 1) * 512], ops_[0:DV, :])

        if stop_stage <= 7:
            nc.sync.dma_start(out_ap[0:DV, 0:512], O1T[:].bitcast(F32))
            return

        # ---------------- out = O1T^T @ WvLT ----------------
        for qo in range(SB):
            fps = sc_ps.tile([P, 1024], F32, name="scps")
            for vc in range(2):
                nc.tensor.matmul(fps[:, vc * 512:(vc + 1) * 512],
                                 O1T[:, qo * P:(qo + 1) * P],
                                 WvLT[:, vc * 512:(vc + 1) * 512],
                                 start=True, stop=True)
            ot = work.tile([P, E], F32, name="outt")
            if qo % 2 == 0:
                nc.vector.tensor_copy(ot[:], fps[:])
            else:
                nc.scalar.copy(ot[:], fps[:])
            nc.sync.dma_start(out_ap[qo * P:(qo + 1) * P, :], ot[:])


def build_nc(reps: int = 1, no_cc=False, no_accum=False, stop_stage=99):
    nc = bacc.Bacc("TRN2", target_bir_lowering=False, debug=False,
                   num_devices=N_CORES)
    aps = {name: nc.dram_tensor(name, shape, F32, kind="ExternalInput").ap()
           for name, shape in IN_SPECS}
    out_ap = nc.dram_tensor("out", [H, E], F32, kind="ExternalOutput").ap()
    with tile.TileContext(nc) as tc:
        if reps == 1:
            _emit(tc, aps, out_ap, no_cc=no_cc, no_accum=no_accum, stop_stage=stop_stage)
        else:
            with tc.For_i(0, reps, 1):
                _emit(tc, aps, out_ap, no_cc=no_cc, no_accum=no_accum, stop_stage=stop_stage)
    nc.compile()
    return nc


def make_in_maps(inputs):
    arrs = {k: np.ascontiguousarray(np.asarray(v, dtype=np.float32))
            for k, v in inputs.items()}
    in_maps = []
    for c in range(N_CORES):
        b, h = divmod(c, 2)
        m = {"x": np.ascontiguousarray(arrs["x"][b, h * H:(h + 1) * H, :]),
             "y": np.ascontiguousarray(arrs["y"][b, h * H:(h + 1) * H, :])}
        for wn in ("Wq", "bq", "Wk", "bk", "WvR", "bvR", "WvL", "bvL"):
            m[wn] = arrs[wn]
        in_maps.append(m)
    return in_maps


def assemble_out(results):
    out = np.empty((B, S, E), dtype=np.float32)
    for c in range(N_CORES):
        b, h = divmod(c, 2)
        out[b, h * H:(h + 1) * H, :] = results[c]["out"]
    return out


_NC = None


def kernel(**inputs) -> np.ndarray:
    global _NC
    if _NC is None:
        _NC = build_nc()
    in_maps = make_in_maps(inputs)
    res = run_bass_kernel_spmd(_NC, in_maps, list(range(N_CORES)))
    return assemble_out(res.results)


# revision 10
# speedup vs baseline: 1.2086x; 1.0108x over previous
"""Trainium2 Bass kernel for nn_CrossAttention_72275709657317 (v2: bf16).

Reference computation (B=4, S=2048, E=1024, D=64):
    Q = x @ Wq.T + bq                      [B,S,D]
    K = y @ Wk.T + bk                      [B,S,D]
    scores = Q @ K.T / sqrt(D)             [B,Sq,Sk]
    attn = softmax(scores, axis=1)         (softmax over the QUERY axis)
    V = (y @ WvR.T + bvR) @ WvL.T + bvL    [B,S,E]
    out = attn @ V                         [B,S,E]

Algebraic restructuring (as v1):
  * V is rank-64 (+bias): attn @ V = (attn @ [VR | 1]) @ [[WvL.T],[bvL]]
  * softmax over q: attn[q,k] = exp(s[q,k])/den[k]; den folded into VR' rows.

v2 changes vs v1 (fp32r):
  * All matmul operands are bf16: fp32 moving operands stream at 2 cycles/col
    on the PE, bf16 at 1 — halves every matmul; bf16 lhsT also gets FWL.
    PSUM accumulation stays fp32; tolerance budget is 2e-2.
  * Inputs cast f32->bf16 during the DMA load (gpsimd SWDGE cast).
  * bf16 PE transposes write bf16 PSUM -> packed 2x DVE copies.
  * K/VR pair exchange is a bf16 AllGather in rank order (no sum-subtract);
    den columns then align across the pair so den is a plain AllReduce.
  * FD=1024 exp over two-bank PSUM tiles; FD=1024 copies.

Sharding: 8 cores -> (batch b = c//2, seq-half h = c%2). Core computes its
q-half of scores/out and its k-half of K/VR; pairs exchange K/VR + den.
"""
import numpy as np

import concourse.bass as bass
import concourse.tile as tile
from concourse import bacc, mybir
from concourse.masks import make_identity
from concourse.bass_utils import run_bass_kernel_spmd

N_CORES = 8
B, S, E, D = 4, 2048, 1024, 64
H = S // 2            # per-core q rows / local k rows
P = 128
EB = E // P           # 8 e-chunks
SB = H // P           # 8 s-chunks
KC = S // P           # 16 global k-chunks
DV = D + 1            # VR plus folded-ones column
F32 = mybir.dt.float32
BF16 = mybir.dt.bfloat16
EXP = mybir.ActivationFunctionType.Exp
ADD = mybir.AluOpType.add
BYPASS = mybir.AluOpType.bypass
GROUPS = [[0, 1], [2, 3], [4, 5], [6, 7]]

IN_SPECS = [
    ("x", [H, E]), ("y", [H, E]),
    ("Wq", [D, E]), ("bq", [D]), ("Wk", [D, E]), ("bk", [D]),
    ("WvR", [D, E]), ("bvR", [D]), ("WvL", [E, D]), ("bvL", [E]),
]


def _emit(tc, aps, out_ap, no_cc=False, no_accum=False, stop_stage=99):
    nc = tc.nc
    from contextlib import ExitStack
    with ExitStack() as ctx:
        const = ctx.enter_context(tc.tile_pool(name="const", bufs=1))
        big = ctx.enter_context(tc.tile_pool(name="big", bufs=1))
        work = ctx.enter_context(tc.tile_pool(name="work", bufs=4))
        tp_ps = ctx.enter_context(tc.tile_pool(name="tp_ps", bufs=2, space="PSUM"))
        mm_ps = ctx.enter_context(tc.tile_pool(name="mm_ps", bufs=2, space="PSUM"))
        sc_ps = ctx.enter_context(tc.tile_pool(name="sc_ps", bufs=2, space="PSUM"))
        dram = ctx.enter_context(tc.tile_pool(name="dram", bufs=1, space="DRAM"))

        # ---------------- identities ----------------
        ident = const.tile([P, P], F32)
        make_identity(nc, ident[:])
        identb = const.tile([P, P], BF16)
        make_identity(nc, identb[:])

        # ---------------- input loads (cast f32->bf16 in DMA) ----------
        Wk_bf = const.tile([D, E], BF16)
        WvR_bf = const.tile([D, E], BF16)
        Wq_bf = const.tile([D, E], BF16)
        WvL_bf = const.tile([P, EB, D], BF16)
        y_bf = big.tile([P, SB, E], BF16, name="y_bf")
        x_bf = big.tile([P, SB, E], BF16, name="x_bf")
        nc.gpsimd.dma_start(Wk_bf[:], aps["Wk"])
        nc.gpsimd.dma_start(WvR_bf[:], aps["WvR"])
        nc.gpsimd.dma_start(Wq_bf[:], aps["Wq"])
        nc.gpsimd.dma_start(WvL_bf[:], aps["WvL"].rearrange("(vo p) d -> p vo d", p=P))
        for sb2 in range(SB // 2):
            nc.gpsimd.dma_start(
                y_bf[:, 2 * sb2:2 * sb2 + 2, :],
                aps["y"][sb2 * 256:(sb2 + 1) * 256, :]
                .rearrange("(c p) e -> p c e", p=P))
        for sb2 in range(SB // 2):
            nc.gpsimd.dma_start(
                x_bf[:, 2 * sb2:2 * sb2 + 2, :],
                aps["x"][sb2 * 256:(sb2 + 1) * 256, :]
                .rearrange("(c p) e -> p c e", p=P))

        # ---------------- biases (f32, one transpose) ----------------
        bias_stage = const.tile([P, D], F32)
        nc.sync.dma_start(bias_stage[0:1, :], aps["bk"].rearrange("(o f) -> o f", o=1))
        nc.sync.dma_start(bias_stage[1:2, :], aps["bvR"].rearrange("(o f) -> o f", o=1))
        nc.sync.dma_start(bias_stage[2:3, :], aps["bq"].rearrange("(o f) -> o f", o=1))
        bias_ps = mm_ps.tile([P, 4 * P], F32, name="mmps")
        nc.tensor.transpose(bias_ps[0:D, 0:P], bias_stage[:], ident[:])
        bias_kv = const.tile([P, 1], F32)
        nc.vector.tensor_copy(bias_kv[0:D, :], bias_ps[0:D, 0:1])
        nc.vector.tensor_copy(bias_kv[D:P, :], bias_ps[0:D, 1:2])
        bias_q = const.tile([D, 1], F32)
        nc.vector.tensor_copy(bias_q[:], bias_ps[0:D, 2:3])

        # ---------------- fused lhsT weights (bf16) ----------------
        # WkvT[:, ei, 0:64] = Wk^T, [:, ei, 64:128] = WvR^T
        WkvT = const.tile([P, EB, P], BF16)
        ps_kv = tp_ps.tile([P, 8 * P], BF16, name="tpb")
        for ei in range(EB):
            nc.tensor.transpose(ps_kv[:, ei * P:ei * P + D],
                                Wk_bf[:, ei * P:(ei + 1) * P], identb[0:D, 0:D])
            nc.tensor.transpose(ps_kv[:, ei * P + D:(ei + 1) * P],
                                WvR_bf[:, ei * P:(ei + 1) * P], identb[0:D, 0:D])
        nc.vector.tensor_copy(WkvT[:], ps_kv[:].rearrange("p (a b) -> p a b", a=EB))

        # WqqT duplicates Wq^T into both halves (M=128 chain)
        WqqT = const.tile([P, EB, P], BF16)
        ps_q = tp_ps.tile([P, 8 * P], BF16, name="tpb")
        for ei in range(EB):
            nc.tensor.transpose(ps_q[:, ei * P:ei * P + D],
                                Wq_bf[:, ei * P:(ei + 1) * P], identb[0:D, 0:D])
        psq3 = ps_q[:].rearrange("p (a b) -> p a b", a=EB)
        nc.vector.tensor_copy(WqqT[:, :, 0:D], psq3[:, :, 0:D])
        nc.vector.tensor_copy(WqqT[:, :, D:P], psq3[:, :, 0:D])

        # WvLT: rows 0:64 = WvL^T, row 64 = bvL
        WvLT = const.tile([DV, E], BF16)
        ps_v = tp_ps.tile([P, 8 * P], BF16, name="tpb")
        for vo in range(EB):
            nc.tensor.transpose(ps_v[0:D, vo * P:(vo + 1) * P],
                                WvL_bf[:, vo, :], identb[:])
        nc.vector.tensor_copy(WvLT[0:D, :], ps_v[0:D, :])
        nc.gpsimd.dma_start(WvLT[D:DV, :], aps["bvL"].rearrange("(o f) -> o f", o=1))

        # ---------------- persistent tiles ----------------
        yT = big.tile([P, EB, H], BF16, name="yT")
        xT = big.tile([P, EB, H], BF16, name="xT")
        QT = big.tile([D, H], BF16, name="QT")
        KTVR_l = big.tile([P, H], BF16, name="KTVR_l")
        KTVR = big.tile([P, 2, H], BF16, name="KTVR")   # rank-ordered pair
        attnT = big.tile([P, KC, H], BF16, name="attnT")
        den = big.tile([P, KC], F32, name="den")
        den2 = big.tile([P, KC, 2], F32, name="den2")
        dsum = big.tile([P, KC], F32, name="dsum")
        r_sb = big.tile([P, KC], F32, name="r_sb")
        VRu = big.tile([P, KC, D], BF16, name="VRu")
        VRp = big.tile([P, KC, P], BF16, name="VRp")
        O1T = big.tile([DV, H], BF16, name="O1T")

        kv_loc = dram.tile([P, H], BF16)
        kv_all = dram.tile([2, P, H], BF16)
        den_dram_h = [dram.tile([P, KC // 2], F32, name=f"den_d{g}") for g in range(2)]
        den_sum_h = [dram.tile([P, KC // 2], F32, name=f"den_s{g}") for g in range(2)]

        nc.gpsimd.memset(VRp[:], 0.0)   # zero-pad cols DV:P for the O1 chain

        if stop_stage <= 0:
            nc.sync.dma_start(out_ap[0:P, 0:P], ident[:])
            return

        # ---------------- transpose + projection blocks ----------------
        def trans_block(src_bf, dstT, sb):
            ps = tp_ps.tile([P, 8 * P], BF16, name="tpb")
            for ei in range(EB):
                nc.tensor.transpose(ps[:, ei * P:(ei + 1) * P],
                                    src_bf[:, sb, ei * P:(ei + 1) * P], identb[:])
            nc.vector.tensor_copy(dstT[:, :, sb * P:(sb + 1) * P],
                                  ps[:].rearrange("p (a b) -> p a b", a=EB))

        def proj_half(wT, srcT, h2, dst_fn):
            ps = mm_ps.tile([P, 512], F32, name="mmps")
            for ei in range(EB):
                nc.tensor.matmul(ps[:], wT[:, ei, :],
                                 srcT[:, ei, h2 * 512:(h2 + 1) * 512],
                                 start=(ei == 0), stop=(ei == EB - 1))
            dst_fn(ps)

        # y path -> KTVR_l
        for sb in range(4):
            trans_block(y_bf, yT, sb)
        proj_half(WkvT, yT, 0,
                  lambda ps: nc.vector.tensor_scalar_add(KTVR_l[:, 0:512], ps[:],
                                                         bias_kv[:]))
        for sb in range(4, SB):
            trans_block(y_bf, yT, sb)
        proj_half(WkvT, yT, 1,
                  lambda ps: nc.vector.tensor_scalar_add(KTVR_l[:, 512:1024], ps[:],
                                                         bias_kv[:]))

        if stop_stage <= 1:
            nc.sync.dma_start(out_ap[0:P, 0:512], KTVR_l[:].bitcast(F32))
            return

        # pair exchange: AllGather K^T/VR^T in rank order
        nc.sync.dma_start(kv_loc[:], KTVR_l[:])
        if no_cc:
            nc.sync.dma_start(kv_all[0], kv_loc[:])
            nc.sync.dma_start(kv_all[1], kv_loc[:])
        else:
            nc.gpsimd.collective_compute(
                "AllGather", BYPASS, replica_groups=GROUPS,
                ins=[kv_loc.opt()], outs=[kv_all.opt()])
        nc.sync.dma_start(KTVR[:], kv_all[:].rearrange("r p h -> p r h"))

        if stop_stage <= 2:
            nc.sync.dma_start(out_ap[0:P, 0:512], KTVR[:, 0, :].bitcast(F32))
            return

        # x path -> QT (overlaps the collective)
        for sb in range(4):
            trans_block(x_bf, xT, sb)
        proj_half(WqqT, xT, 0,
                  lambda ps: nc.vector.tensor_scalar_add(QT[:, 0:512], ps[0:D, :],
                                                         bias_q[:]))
        for sb in range(4, SB):
            trans_block(x_bf, xT, sb)
        proj_half(WqqT, xT, 1,
                  lambda ps: nc.vector.tensor_scalar_add(QT[:, 512:1024], ps[0:D, :],
                                                         bias_q[:]))

        if stop_stage <= 3:
            nc.sync.dma_start(out_ap[0:D, 0:512], QT[:].bitcast(F32))
            return

        # ---------------- scoresT + exp + den partials ----------------
        r_bf = big.tile([P, KC], BF16, name="r_bf")

        def den_exchange(g):
            # den half g: dump, pairwise AllReduce, read back, recip, VR'
            lo, hi = g * (KC // 2), (g + 1) * (KC // 2)
            nc.sync.dma_start(den_dram_h[g][:], den[:, lo:hi])
            if no_cc:
                nc.sync.dma_start(den_sum_h[g][:], den_dram_h[g][:])
            else:
                nc.gpsimd.collective_compute(
                    "AllReduce", ADD, replica_groups=GROUPS,
                    ins=[den_dram_h[g].opt()], outs=[den_sum_h[g].opt()])
            nc.sync.dma_start(dsum[:, lo:hi], den_sum_h[g][:])
            nc.vector.reciprocal(r_sb[:, lo:hi], dsum[:, lo:hi])
            for kc in range(lo, hi):
                nc.vector.tensor_scalar_mul(VRp[:, kc, 0:D], VRu[:, kc, :],
                                            r_sb[:, kc:kc + 1])
            nc.vector.tensor_copy(r_bf[:, lo:hi], r_sb[:, lo:hi])
            nc.vector.tensor_copy(
                VRp[:, lo:hi, D:DV],
                r_bf[:, lo:hi].rearrange("p (a b) -> p a b", a=KC // 2))

        for kc in range(KC):
            r, j = divmod(kc, SB)
            sps = sc_ps.tile([P, 1024], F32, name="scps")
            for qc in range(2):
                nc.tensor.matmul(sps[:, qc * 512:(qc + 1) * 512],
                                 KTVR[0:D, r, j * P:(j + 1) * P],
                                 QT[:, qc * 512:(qc + 1) * 512],
                                 start=True, stop=True)
            nc.scalar.activation(attnT[:, kc, :], sps[:], EXP, scale=0.125,
                                 accum_out=den[:, kc:kc + 1])
            if kc == 7:
                # VR^T -> VRu early (PE slots after the scores matmuls)
                for g in range(2):
                    ps = tp_ps.tile([P, 8 * P], BF16, name="tpb")
                    for jj in range(SB):
                        nc.tensor.transpose(ps[:, jj * P:(jj + 1) * P],
                                            KTVR[:, g, jj * P:(jj + 1) * P],
                                            identb[:])
                    nc.vector.tensor_copy(
                        VRu[:, 8 * g:8 * g + 8, :],
                        ps[:].rearrange("p (a b) -> p a b", a=SB)[:, :, D:P])
                den_exchange(0)
        den_exchange(1)

        if stop_stage <= 4:
            nc.sync.dma_start(out_ap[0:P, 0:512], attnT[:, 0, :].bitcast(F32))
            return
        if stop_stage <= 5:
            nc.sync.dma_start(out_ap[0:P, 0:KC], r_sb[:])
            return
        if stop_stage <= 6:
            nc.sync.dma_start(out_ap[0:P, 0:64], VRp[:, 0, :].bitcast(F32))
            return

        # ---------------- O1T = VR'^T @ attnT; out = O1T^T @ WvLT --------
        def out_rows(qo):
            fps = sc_ps.tile([P, 1024], F32, name="scps")
            for vc in range(2):
                nc.tensor.matmul(fps[:, vc * 512:(vc + 1) * 512],
                                 O1T[:, qo * P:(qo + 1) * P],
                                 WvLT[:, vc * 512:(vc + 1) * 512],
                                 start=True, stop=True)
            ot = work.tile([P, E], F32, name="outt")
            if qo % 2 == 0:
                nc.vector.tensor_copy(ot[:], fps[:])
            else:
                nc.scalar.copy(ot[:], fps[:])
            nc.sync.dma_start(out_ap[qo * P:(qo + 1) * P, :], ot[:])

        ops_ = [mm_ps.tile([P, 512], F32, name="mmps") for _ in range(2)]
        for qc in range(2):          # first halves: only need VRp 0:8
            for kc in range(KC // 2):
                nc.tensor.matmul(ops_[qc][:], VRp[:, kc, :],
                                 attnT[:, kc, qc * 512:(qc + 1) * 512],
                                 start=(kc == 0), stop=False)
        for qc in range(2):
            for kc in range(KC // 2, KC):
                nc.tensor.matmul(ops_[qc][:], VRp[:, kc, :],
                                 attnT[:, kc, qc * 512:(qc + 1) * 512],
                                 start=False, stop=(kc == KC - 1))
            nc.scalar.copy(O1T[:, qc * 512:(qc + 1) * 512], ops_[qc][0:DV, :])
            if stop_stage > 7:
                for qo in range(qc * 4, qc * 4 + 4):
                    out_rows(qo)

        if stop_stage <= 7:
            nc.sync.dma_start(out_ap[0:DV, 0:512], O1T[:].bitcast(F32))
            return


def build_nc(reps: int = 1, no_cc=False, no_accum=False, stop_stage=99):
    nc = bacc.Bacc("TRN2", target_bir_lowering=False, debug=False,
                   num_devices=N_CORES)
    aps = {name: nc.dram_tensor(name, shape, F32, kind="ExternalInput").ap()
           for name, shape in IN_SPECS}
    out_ap = nc.dram_tensor("out", [H, E], F32, kind="ExternalOutput").ap()
    with tile.TileContext(nc) as tc:
        if reps == 1:
            _emit(tc, aps, out_ap, no_cc=no_cc, no_accum=no_accum, stop_stage=stop_stage)
        else:
            with tc.For_i(0, reps, 1):
                _emit(tc, aps, out_ap, no_cc=no_cc, no_accum=no_accum, stop_stage=stop_stage)
    nc.compile()
    return nc


def make_in_maps(inputs):
    arrs = {k: np.ascontiguousarray(np.asarray(v, dtype=np.float32))
            for k, v in inputs.items()}
    in_maps = []
    for c in range(N_CORES):
        b, h = divmod(c, 2)
        m = {"x": np.ascontiguousarray(arrs["x"][b, h * H:(h + 1) * H, :]),
             "y": np.ascontiguousarray(arrs["y"][b, h * H:(h + 1) * H, :])}
        for wn in ("Wq", "bq", "Wk", "bk", "WvR", "bvR", "WvL", "bvL"):
            m[wn] = arrs[wn]
        in_maps.append(m)
    return in_maps


def assemble_out(results):
    out = np.empty((B, S, E), dtype=np.float32)
    for c in range(N_CORES):
        b, h = divmod(c, 2)
        out[b, h * H:(h + 1) * H, :] = results[c]["out"]
    return out


_NC = None


def kernel(**inputs) -> np.ndarray:
    global _NC
    if _NC is None:
        _NC = build_nc()
    in_maps = make_in_maps(inputs)
    res = run_bass_kernel_spmd(_NC, in_maps, list(range(N_CORES)))
    return assemble_out(res.results)
